# revision 1
# baseline (speedup 1.0000x reference)
"""Multi-head attention (S=2048, B=2, D=1024, H=16) on 8 Trainium2 NeuronCores.

Sharding: tensor-parallel over heads. Each core computes 2 heads end-to-end
(QKV projections restricted to its 128 output dims, attention, and the
row-parallel slice of the output projection). The host sums the 8 partial
outputs (row-parallel Wo ==> partial sums) and adds bo.

On-device compute is fp16 with fp32 PSUM accumulation. The softmax scale and
the (additive log-)mask fold into the exp activation; the softmax denominator
comes for free from a ones-column appended to V.
"""

import math

import numpy as np

S, B, D, H = 2048, 2, 1024, 16
DK = D // H  # 64
NCORES = 8
HLOC = H // NCORES        # heads per core = 2
DLOC = HLOC * DK          # local output dims per core = 128
T = S * B                 # tokens = 4096
KT = D // 128             # contraction tiles = 8
NQC = S // 512            # query chunks per batch = 4
NKB = S // 128            # key blocks = 16
NTT = S // 128            # token tiles per batch = 16
SCALE = 1.0 / math.sqrt(DK)

_prog_cache = {}


def _build(masked: bool):
    import concourse.mybir as mybir
    import concourse.tile as tile
    from concourse import bacc

    f16 = mybir.dt.float16
    f32 = mybir.dt.float32
    EXP = mybir.ActivationFunctionType.Exp
    MUL = mybir.AluOpType.mult
    ADD = mybir.AluOpType.add

    nc = bacc.Bacc("TRN2", target_bir_lowering=False, debug=False)

    def din(name, shape, dt=f16):
        return nc.dram_tensor(name, shape, dt, kind="ExternalInput").ap()

    xq = din("xq", [D, B, S])          # query^T
    xk = din("xk", [D, B, S])          # key^T
    xv = din("xv", [D, B, S])          # value^T
    # per-core projection weights, host-prearranged to [p, kt*m] so the
    # DMA is 128 partitions x 2KB contiguous (wq_arr[p, kt, m] = Wq[hs+m, kt*128+p])
    wq = din("wq", [128, KT * DLOC])
    wk = din("wk", [128, KT * DLOC])
    wv = din("wv", [128, KT * DLOC])
    wo = din("wo", [DLOC, D])          # Wo[:, hs:hs+128].T
    bq = din("bq", [DLOC], f32)
    bk = din("bk", [DLOC], f32)
    bv = din("bv", [DLOC], f32)
    mb = din("mb", [S], f32)           # additive mask bias per key (0 / -1e30)
    out = nc.dram_tensor("out", [S, B, D], f32, kind="ExternalOutput").ap()
    import os
    _dbg = bool(os.environ.get("KDBG"))
    if _dbg:
        dbg_qT = nc.dram_tensor("dbg_qT", [DLOC, B, S], f16, kind="ExternalOutput").ap()
        dbg_kT = nc.dram_tensor("dbg_kT", [DLOC, B, S], f16, kind="ExternalOutput").ap()
        dbg_vv = nc.dram_tensor("dbg_vv", [128, B, HLOC, NKB, 68], f16, kind="ExternalOutput").ap()
        dbg_cn = nc.dram_tensor("dbg_cn", [DLOC, B, S], f16, kind="ExternalOutput").ap()

    with tile.TileContext(nc) as tc:
        with (
            tc.tile_pool(name="wsb", bufs=1) as wsb,
            tc.tile_pool(name="xsb", bufs=8) as xsb,
            tc.tile_pool(name="qkv", bufs=1) as qkv,
            tc.tile_pool(name="esb", bufs=6) as esb,
            tc.tile_pool(name="nrm", bufs=3) as nrm,
            tc.tile_pool(name="osb", bufs=4) as osb,
            tc.tile_pool(name="pj", bufs=2, space="PSUM") as pj,
            tc.tile_pool(name="psc", bufs=2, space="PSUM") as psc,
            tc.tile_pool(name="pcx", bufs=1, space="PSUM") as pcx,
        ):
            # ---- constants / weights -------------------------------------
            w_sb = {}
            for name, ap in (("wq", wq), ("wk", wk), ("wv", wv)):
                t = wsb.tile([128, KT, DLOC], f16, tag=name)
                nc.sync.dma_start(out=t, in_=ap.rearrange("p (kt m) -> p kt m", kt=KT))
                w_sb[name] = t
            wo_sb = wsb.tile([DLOC, D], f16, tag="wo")
            nc.sync.dma_start(out=wo_sb, in_=wo)
            bq_sb = wsb.tile([DLOC, 1], f32, tag="bq")
            nc.sync.dma_start(out=bq_sb, in_=bq.unsqueeze(1))
            bk_sb = wsb.tile([DLOC, 1], f32, tag="bk")
            nc.sync.dma_start(out=bk_sb, in_=bk.unsqueeze(1))
            bv_row = wsb.tile([1, DLOC], f32, tag="bv_row")
            nc.sync.dma_start(out=bv_row, in_=bv.unsqueeze(0))
            bv_bc = wsb.tile([128, DLOC], f32, tag="bv_bc")
            nc.gpsimd.partition_broadcast(bv_bc, bv_row)
            mb_sb = wsb.tile([128, NKB], f32, tag="mb")
            nc.sync.dma_start(out=mb_sb, in_=mb.rearrange("(kb p) -> p kb", p=128))

            # persistent per-batch activations
            qT = [qkv.tile([DLOC, S], f16, tag=f"qT{b}", name=f"qT{b}") for b in range(B)]
            kT = [qkv.tile([DLOC, S], f16, tag=f"kT{b}", name=f"kT{b}") for b in range(B)]
            # V per (head, key-block): [keys=128, 65] with ones in col 64
            vv = [qkv.tile([128, HLOC, NKB, 68], f16, tag=f"vv{b}", name=f"vv{b}") for b in range(B)]
            for b in range(B):
                nc.vector.memset(vv[b], 0.0)
                nc.vector.memset(vv[b][:, :, :, 64:65], 1.0)
            ctxn = [qkv.tile([DLOC, S], f16, tag=f"ctxn{b}", name=f"ctxn{b}") for b in range(B)]

            def load_x(ap, name, b, eng=None):
                eng = eng or nc.sync
                ts = []
                for kt in range(KT):
                    t = xsb.tile([128, S], f16, tag=f"x{name}", name=f"x{name}{kt}")
                    half = S // 2
                    for i in range(2):
                        eng.dma_start(
                            out=t[:, i * half:(i + 1) * half],
                            in_=ap[kt * 128:(kt + 1) * 128, b,
                                   i * half:(i + 1) * half])
                    ts.append(t)
                return ts

            def proj_qk_chunk(b, which, xt, qc):
                w, bias, dst = (("wq", bq_sb, qT) if which == "q"
                                else ("wk", bk_sb, kT))
                ps = pj.tile([DLOC, 512], f32, tag="pj", name="ps")
                sl = slice(qc * 512, (qc + 1) * 512)
                for kt in range(KT):
                    nc.tensor.matmul(ps, w_sb[w][:, kt, :], xt[kt][:, sl],
                                     start=(kt == 0), stop=(kt == KT - 1))
                nc.vector.tensor_scalar(out=dst[b][:, sl], in0=ps,
                                        scalar1=bias, scalar2=None, op0=ADD)

            def proj_qk(b, which, xt):
                for qc in range(NQC):
                    proj_qk_chunk(b, which, xt, qc)

            def proj_v_tt(b, xt, tts):
                for tt in tts:
                    ps = pj.tile([128, DLOC], f32, tag="pj", name="ps")
                    sl = slice(tt * 128, (tt + 1) * 128)
                    for kt in range(KT):
                        nc.tensor.matmul(ps, xt[kt][:, sl], w_sb["wv"][:, kt, :],
                                         start=(kt == 0), stop=(kt == KT - 1))
                    for h in range(HLOC):
                        nc.vector.tensor_tensor(
                            out=vv[b][:, h, tt, 0:64],
                            in0=ps[:, h * 64:(h + 1) * 64],
                            in1=bv_bc[:, h * 64:(h + 1) * 64], op=ADD)

            def outproj_tt(b, tts, on_scalar=False):
                for tt in tts:
                    tsl = slice(tt * 128, (tt + 1) * 128)
                    for eh in range(2):
                        po = pj.tile([128, 512], f32, tag="pj", name="po")
                        nc.tensor.matmul(po, ctxn[b][:, tsl],
                                         wo_sb[:, eh * 512:(eh + 1) * 512],
                                         start=True, stop=True)
                        oc = osb.tile([128, 512], f32, tag="oc", name="oc")
                        if on_scalar:
                            nc.scalar.copy(oc, po)
                        else:
                            nc.vector.tensor_copy(oc, po)
                        nc.gpsimd.dma_start(
                            out=out[tsl, b, eh * 512:(eh + 1) * 512], in_=oc)

            def attn_qc(b, qc, injects=()):
                """Attention for one (b, qc); each inject thunk is emitted
                after a score batch so independent PE work (stage A of the
                next batch, V projections, prior outproj) spreads through
                the stream without starving ACT."""
                injects = list(injects)
                qsl = slice(qc * 512, (qc + 1) * 512)
                pctx = [pcx.tile([65, 512], f32, tag=f"cx{h}", name=f"cx{h}")
                        for h in range(HLOC)]
                escore = {}

                def scores(kbp):
                    psco = [psc.tile([128, 1024], f32, tag="sc", name="sc")
                            for _ in range(HLOC)]
                    for i in range(2):
                        kb = kbp * 2 + i
                        ksl = slice(kb * 128, (kb + 1) * 128)
                        for h in range(HLOC):
                            hsl = slice(h * 64, (h + 1) * 64)
                            nc.tensor.matmul(
                                psco[h][:, i * 512:(i + 1) * 512],
                                kT[b][hsl, ksl], qT[b][hsl, qsl],
                                start=True, stop=True,
                                tile_position=(h * 64, 0))
                    return psco

                def exp_ctx(kbp, psco):
                    for h in range(HLOC):
                        et = esb.tile([128, 1024], f16, tag="e", name="et")
                        if masked:
                            for i in range(2):
                                kb = kbp * 2 + i
                                nc.scalar.activation(
                                    et[:, i * 512:(i + 1) * 512],
                                    psco[h][:, i * 512:(i + 1) * 512],
                                    EXP, bias=mb_sb[:, kb:kb + 1], scale=SCALE)
                        else:
                            nc.scalar.activation(et, psco[h], EXP, scale=SCALE)
                        escore[h] = et
                    for i in range(2):
                        kb = kbp * 2 + i
                        for h in range(HLOC):
                            nc.tensor.matmul(
                                pctx[h], vv[b][:, h, kb, 0:65],
                                escore[h][:, i * 512:(i + 1) * 512],
                                start=(kb == 0), stop=(kb == NKB - 1))

                prev = scores(0)
                cur = scores(1)
                if injects:
                    injects.pop(0)()
                for kbp in range(2, NKB // 2):
                    nxt = scores(kbp)
                    if injects:
                        injects.pop(0)()
                    exp_ctx(kbp - 2, prev)
                    prev, cur = cur, nxt
                exp_ctx(NKB // 2 - 2, prev)
                exp_ctx(NKB // 2 - 1, cur)
                while injects:
                    injects.pop(0)()

                for h in range(HLOC):
                    hsl = slice(h * 64, (h + 1) * 64)
                    cd = nrm.tile([64, 512], f32, tag="cd", name="cd")
                    nc.vector.tensor_copy(cd, pctx[h][0:64, :])
                    cl = nrm.tile([1, 512], f32, tag="cl", name="cl")
                    nc.vector.tensor_copy(cl, pctx[h][64:65, :])
                    # reciprocal_approx_fast requires base partition 0 input
                    rl = nrm.tile([1, 512], f32, tag="rl", name="rl")
                    nc.vector.reciprocal_approx_fast(rl, cl)
                    rl_bc = nrm.tile([64, 512], f32, tag="rlb", name="rlb")
                    nc.gpsimd.partition_broadcast(rl_bc, rl)
                    nc.vector.tensor_tensor(out=ctxn[b][hsl, qsl],
                                            in0=cd, in1=rl_bc, op=MUL)

            def nothing():
                pass

            # stage A head for b=0: K fully + Q chunk 0; Q rest injected.
            xk_t = load_x(xk, "k", 0)
            xq_t = load_x(xq, "q", 0, eng=nc.scalar)
            # PE warmup while the activation DMAs land: junk matmuls on the
            # small weight tiles keep the HAM clock at 2.4GHz.
            for wu in range(40):
                jp = psc.tile([128, 512], f32, tag="sc", name="jp")
                nc.tensor.matmul(jp, w_sb["wq"][:, wu % 8, :],
                                 w_sb["wk"][:, (wu % 2) * 4:(wu % 2) * 4 + 4, :],
                                 start=True, stop=True)
            proj_qk(0, "k", xk_t)
            proj_qk_chunk(0, "q", xq_t, 0)
            xv_t = load_x(xv, "v", 0)

            x2 = {}
            attn_qc(0, 0, [
                lambda: (proj_qk_chunk(0, "q", xq_t, 1),
                         proj_v_tt(0, xv_t, range(0, 4))),
                lambda: proj_v_tt(0, xv_t, range(4, 8)),
                lambda: (proj_qk_chunk(0, "q", xq_t, 2),
                         proj_v_tt(0, xv_t, range(8, 12))),
                lambda: proj_v_tt(0, xv_t, range(12, 16)),
                lambda: proj_qk_chunk(0, "q", xq_t, 3),
            ])
            attn_qc(0, 1, [
                lambda: x2.update(k=load_x(xk, "k", 1)),
                lambda: proj_qk_chunk(1, "k", x2["k"], 0),
                lambda: proj_qk_chunk(1, "k", x2["k"], 1),
                lambda: proj_qk_chunk(1, "k", x2["k"], 2),
                lambda: proj_qk_chunk(1, "k", x2["k"], 3),
            ])
            attn_qc(0, 2, [
                lambda: x2.update(q=load_x(xq, "q", 1)),
                lambda: proj_qk_chunk(1, "q", x2["q"], 0),
                lambda: proj_qk_chunk(1, "q", x2["q"], 1),
                lambda: outproj_tt(0, [0, 1]),
                lambda: proj_qk_chunk(1, "q", x2["q"], 2),
                lambda: proj_qk_chunk(1, "q", x2["q"], 3),
            ])
            attn_qc(0, 3, [
                lambda: x2.update(v=load_x(xv, "v", 1)),
                lambda: proj_v_tt(1, x2["v"], range(0, 4)),
                lambda: proj_v_tt(1, x2["v"], range(4, 8)),
                lambda: outproj_tt(0, [2, 3]),
                lambda: proj_v_tt(1, x2["v"], range(8, 12)),
                lambda: proj_v_tt(1, x2["v"], range(12, 16)),
            ])
            attn_qc(1, 0, [
                lambda: outproj_tt(0, [4, 5]),
                lambda: outproj_tt(0, [6, 7]),
                lambda: outproj_tt(0, [8, 9]),
                lambda: outproj_tt(0, [10, 11]),
                lambda: outproj_tt(0, [12, 13]),
                lambda: outproj_tt(0, [14, 15]),
            ])
            attn_qc(1, 1, [
                lambda: outproj_tt(1, [0, 1]),
                lambda: outproj_tt(1, [2, 3]),
            ])
            attn_qc(1, 2, [
                lambda: outproj_tt(1, [4, 5]),
                lambda: outproj_tt(1, [6, 7]),
            ])
            attn_qc(1, 3, [
                lambda: outproj_tt(1, [8, 9]),
                lambda: outproj_tt(1, [10, 11]),
            ])
            outproj_tt(1, [12, 13], on_scalar=True)
            outproj_tt(1, [14, 15], on_scalar=True)
            if _dbg:
                for b in range(B):
                    nc.sync.dma_start(out=dbg_qT[:, b, :], in_=qT[b])
                    nc.sync.dma_start(out=dbg_kT[:, b, :], in_=kT[b])
                    nc.sync.dma_start(out=dbg_vv[:, b], in_=vv[b])
                    nc.sync.dma_start(out=dbg_cn[:, b, :], in_=ctxn[b])


    nc.compile()
    return nc


def _get_prog(masked: bool):
    key = masked
    if key not in _prog_cache:
        _prog_cache[key] = _build(masked)
    return _prog_cache[key]


def kernel(query, key, value, mask, Wq, bq, Wk, bk, Wv, bv, Wo, bo):
    from concourse.bass_utils import run_bass_kernel_spmd

    query = np.asarray(query)
    key = np.asarray(key)
    value = np.asarray(value)
    mask = np.asarray(mask)
    Wq, bq = np.asarray(Wq), np.asarray(bq)
    Wk, bk = np.asarray(Wk), np.asarray(bk)
    Wv, bv = np.asarray(Wv), np.asarray(bv)
    Wo, bo = np.asarray(Wo), np.asarray(bo)

    masked = not bool(mask.all())
    nc = _get_prog(masked)

    def t16(x):  # [S, B, D] -> contiguous [D, B, S] fp16
        return np.ascontiguousarray(x.transpose(2, 1, 0).astype(np.float16))

    def warr(W, hs):  # [128, KT*128]: row p = concat_kt W[hs+m, kt*128+p]
        wt = W[hs:hs + DLOC, :].T.astype(np.float16)       # [kt*128+p, m]
        return np.ascontiguousarray(
            wt.reshape(KT, 128, DLOC).transpose(1, 0, 2).reshape(128, KT * DLOC))

    xq, xk, xv = t16(query), t16(key), t16(value)
    mb = np.where(mask.reshape(S), 0.0, -1e30).astype(np.float32)

    in_maps = []
    for c in range(NCORES):
        hs = c * DLOC
        in_maps.append({
            "xq": xq, "xk": xk, "xv": xv,
            "wq": warr(Wq, hs),
            "wk": warr(Wk, hs),
            "wv": warr(Wv, hs),
            "wo": np.ascontiguousarray(Wo[:, hs:hs + DLOC].T.astype(np.float16)),
            "bq": bq[hs:hs + DLOC].astype(np.float32),
            "bk": bk[hs:hs + DLOC].astype(np.float32),
            "bv": bv[hs:hs + DLOC].astype(np.float32),
            "mb": mb,
        })

    res = run_bass_kernel_spmd(nc, in_maps, core_ids=list(range(NCORES)))
    acc = res.results[0]["out"].astype(np.float64)
    for c in range(1, NCORES):
        acc += res.results[c]["out"]
    acc += bo.astype(np.float64)
    return acc.astype(np.float32)



# revision 9
# speedup vs baseline: 1.1896x; 1.1896x over previous
"""Multi-head attention (S=2048, B=2, D=1024, H=16) on 8 Trainium2 NeuronCores.

Sharding: tensor-parallel over heads. Each core computes 2 heads end-to-end
(QKV projections restricted to its 128 output dims, attention, and the
row-parallel slice of the output projection). The host sums the 8 partial
outputs (row-parallel Wo ==> partial sums) and adds bo.

All compute is fp16 with fp32 PSUM accumulation (fp8 was measured to cost
~8% output error because attention concentrates on few keys, so quantization
noise does not average out). The softmax denominator comes for free from a
ones-column appended to V; the 1/den normalization fuses into the mandatory
PSUM->SBUF context copy.

Pipeline: the Scalar engine (softmax EXP over 16.8M scores, ~1.3us per
[128,1024] tile) is the roofline. Per key block: two head-packed score
matmuls (tile_position row packing, adjacent issue so they overlap in the
PE array) -> one EXP covering both heads -> PV accumulation deferred a full
q-chunk so DMA/projection stalls never starve the ACT stream. Projections
and the output projection are injected between attention steps as PE filler.
"""

import math

import numpy as np

S, B, D, H = 2048, 2, 1024, 16
DK = D // H               # 64
NCORES = 8
HLOC = H // NCORES        # heads per core = 2
DLOC = HLOC * DK          # local output dims per core = 128
T = S * B                 # tokens = 4096
KT = D // 128             # contraction tiles = 8
NQC = S // 512            # query chunks per batch = 4
NKB = S // 128            # key blocks = 16
NTT = S // 128            # token tiles per batch = 16
SCALE = 1.0 / math.sqrt(DK)

_prog_cache = {}


def _build(masked: bool):
    import concourse.mybir as mybir
    import concourse.tile as tile
    from concourse import bacc

    f16 = mybir.dt.float16
    f32 = mybir.dt.float32
    EXP = mybir.ActivationFunctionType.Exp
    MUL = mybir.AluOpType.mult
    ADD = mybir.AluOpType.add

    nc = bacc.Bacc("TRN2", target_bir_lowering=False, debug=False)

    def din(name, shape, dt=f16):
        return nc.dram_tensor(name, shape, dt, kind="ExternalInput").ap()

    xq = din("xq", [D, B, S])          # query^T
    xk = din("xk", [D, B, S])          # key^T
    xv = din("xv", [D, B, S])          # value^T
    # per-core projection weights, host-prearranged to [p, kt*m] so the
    # DMA is 128 partitions x 2KB contiguous (wq_arr[p, kt, m] = Wq[hs+m, kt*128+p])
    wq = din("wq", [128, KT * DLOC])
    wk = din("wk", [128, KT * DLOC])
    wv = din("wv", [128, KT * DLOC])
    wo = din("wo", [DLOC, D])          # Wo[:, hs:hs+128].T
    bq = din("bq", [DLOC], f32)
    bk = din("bk", [DLOC], f32)
    bv = din("bv", [DLOC], f32)
    mb = din("mb", [S], f32)           # additive mask bias per key (0 / -1e30)
    out = nc.dram_tensor("out", [S, B, D], f16, kind="ExternalOutput").ap()

    with tile.TileContext(nc) as tc:
        with (
            tc.tile_pool(name="wsb", bufs=1) as wsb,
            tc.tile_pool(name="xsb", bufs=8) as xsb,
            tc.tile_pool(name="qkv", bufs=1) as qkv,
            tc.tile_pool(name="esb", bufs=20) as esb,
            tc.tile_pool(name="nrm", bufs=2) as nrm,
            tc.tile_pool(name="osb", bufs=4) as osb,
            tc.tile_pool(name="pj", bufs=2, space="PSUM") as pj,
            tc.tile_pool(name="psc", bufs=2, space="PSUM") as psc,
            tc.tile_pool(name="pcx", bufs=1, space="PSUM") as pcx,
        ):
            # ---- constants / weights -------------------------------------
            w_sb = {}
            for name, ap in (("wq", wq), ("wk", wk), ("wv", wv)):
                t = wsb.tile([128, KT, DLOC], f16, tag=name)
                nc.sync.dma_start(out=t, in_=ap.rearrange("p (kt m) -> p kt m", kt=KT))
                w_sb[name] = t
            wo_sb = wsb.tile([DLOC, D], f16, tag="wo")
            nc.sync.dma_start(out=wo_sb, in_=wo)
            bq_sb = wsb.tile([DLOC, 1], f32, tag="bq")
            nc.sync.dma_start(out=bq_sb, in_=bq.unsqueeze(1))
            bk_sb = wsb.tile([DLOC, 1], f32, tag="bk")
            nc.sync.dma_start(out=bk_sb, in_=bk.unsqueeze(1))
            bv_row = wsb.tile([1, DLOC], f32, tag="bv_row")
            nc.sync.dma_start(out=bv_row, in_=bv.unsqueeze(0))
            bv_bc = wsb.tile([128, DLOC], f32, tag="bv_bc")
            nc.gpsimd.partition_broadcast(bv_bc, bv_row)
            mb_sb = wsb.tile([128, NKB], f32, tag="mb")
            nc.sync.dma_start(out=mb_sb, in_=mb.rearrange("(kb p) -> p kb", p=128))
            # preload the exp table set while input DMAs are in flight
            warm_e = wsb.tile([1, 1], f16, tag="warm_e")
            nc.scalar.activation(warm_e, mb_sb[0:1, 0:1], EXP, scale=1.0)

            # persistent per-batch activations
            qT = [qkv.tile([DLOC, S], f16, tag=f"qT{b}", name=f"qT{b}") for b in range(B)]
            kT = [qkv.tile([DLOC, S], f16, tag=f"kT{b}", name=f"kT{b}") for b in range(B)]
            # V per (key-block, head): [keys=128, 68] with ones in col 64
            # (ctx on PSUM partitions 0:64, softmax denominator on 64)
            vv = [qkv.tile([128, NKB, HLOC, 68], f16, tag=f"vv{b}", name=f"vv{b}")
                  for b in range(B)]
            for b in range(B):
                nc.vector.memset(vv[b], 0.0)
                nc.vector.memset(vv[b][:, :, :, 64:65], 1.0)
            ctxn = [qkv.tile([DLOC, S], f16, tag=f"ctxn{b}", name=f"ctxn{b}") for b in range(B)]

            def load_x(ap, name, b, mode):
                """mode: 'sync'/'scalar' = whole tensor on one HWDGE queue,
                'dual' = half0 on sync + half1 on scalar. Emits half0 of all
                kt tiles first so the first proj chunks unblock early."""
                ts = [xsb.tile([128, S], f16, tag=f"x{name}", name=f"x{name}{kt}")
                      for kt in range(KT)]
                half = S // 2
                for i in range(2):
                    if mode == "dual":
                        eng = (nc.sync, nc.scalar)[i]
                    else:
                        eng = nc.sync if mode == "sync" else nc.scalar
                    for kt in range(KT):
                        eng.dma_start(
                            out=ts[kt][:, i * half:(i + 1) * half],
                            in_=ap[kt * 128:(kt + 1) * 128, b,
                                   i * half:(i + 1) * half])
                return ts

            def proj_qk_chunk(b, which, xt, qc):
                w, bias, dst = (("wq", bq_sb, qT) if which == "q"
                                else ("wk", bk_sb, kT))
                ps = pj.tile([DLOC, 512], f32, tag="pj", name="ps")
                sl = slice(qc * 512, (qc + 1) * 512)
                for kt in range(KT):
                    nc.tensor.matmul(ps, w_sb[w][:, kt, :], xt[kt][:, sl],
                                     start=(kt == 0), stop=(kt == KT - 1))
                nc.vector.tensor_scalar(out=dst[b][:, sl], in0=ps,
                                        scalar1=bias, scalar2=None, op0=ADD)

            def proj_v_tt(b, xt, tts):
                for tt in tts:
                    ps = pj.tile([128, DLOC], f32, tag="pj", name="ps")
                    sl = slice(tt * 128, (tt + 1) * 128)
                    for kt in range(KT):
                        nc.tensor.matmul(ps, xt[kt][:, sl], w_sb["wv"][:, kt, :],
                                         start=(kt == 0), stop=(kt == KT - 1))
                    for h in range(HLOC):
                        nc.vector.tensor_tensor(
                            out=vv[b][:, tt, h, 0:64],
                            in0=ps[:, h * 64:(h + 1) * 64],
                            in1=bv_bc[:, h * 64:(h + 1) * 64], op=ADD)

            def outproj_tt(b, tts, on_scalar=False):
                for tt in tts:
                    tsl = slice(tt * 128, (tt + 1) * 128)
                    for eh in range(2):
                        po = pj.tile([128, 512], f32, tag="pj", name="po")
                        nc.tensor.matmul(po, ctxn[b][:, tsl],
                                         wo_sb[:, eh * 512:(eh + 1) * 512],
                                         start=True, stop=True)
                        oc = osb.tile([128, 512], f16, tag="oc", name="oc")
                        if on_scalar:
                            nc.scalar.copy(oc, po)
                        else:
                            nc.vector.tensor_copy(oc, po)
                        nc.gpsimd.dma_start(
                            out=out[tsl, b, eh * 512:(eh + 1) * 512], in_=oc)

            def attn_qc(b, qc, s_inj=(), pv_inj=()):
                """One 512-wide query chunk: 16 score+exp steps feed the ACT
                stream, then 16 PV accumulation steps consume the exp tiles.
                Injected thunks run between steps as PE filler; score/PV
                matmuls are emitted so PSUM WAR waits never head-block ready
                work."""
                s_inj, pv_inj = list(s_inj), list(pv_inj)
                qsl = slice(qc * 512, (qc + 1) * 512)
                pctx = [pcx.tile([65, 512], f32, tag=f"cx{h}", name=f"cx{h}")
                        for h in range(HLOC)]
                ets = {}

                for kb in range(NKB):
                    t = psc.tile([128, HLOC, 512], f32, tag="sc", name="sc")
                    ksl = slice(kb * 128, (kb + 1) * 128)
                    for h in range(HLOC):
                        hsl = slice(h * 64, (h + 1) * 64)
                        nc.tensor.matmul(t[:, h, :], kT[b][hsl, ksl],
                                         qT[b][hsl, qsl], start=True, stop=True,
                                         tile_position=(h * 64, 0))
                    et = esb.tile([128, HLOC, 512], f16, tag="e", name="et")
                    if masked:
                        nc.scalar.activation(et, t, EXP,
                                             bias=mb_sb[:, kb:kb + 1], scale=SCALE)
                    else:
                        nc.scalar.activation(et, t, EXP, scale=SCALE)
                    ets[kb] = et
                    if s_inj:
                        s_inj.pop(0)()

                for kb in range(NKB):
                    et = ets.pop(kb)
                    for h in range(HLOC):
                        nc.tensor.matmul(pctx[h], vv[b][:, kb, h, 0:65],
                                         et[:, h, :],
                                         start=(kb == 0), stop=(kb == NKB - 1))
                    if pv_inj:
                        pv_inj.pop(0)()
                while pv_inj:
                    pv_inj.pop(0)()

                for h in range(HLOC):
                    hsl = slice(h * 64, (h + 1) * 64)
                    # reciprocal_approx_fast requires base partition 0 input
                    cl = nrm.tile([1, 512], f32, tag="cl", name="cl")
                    nc.vector.tensor_copy(cl, pctx[h][64:65, :])
                    rl = nrm.tile([1, 512], f32, tag="rl", name="rl")
                    nc.vector.reciprocal_approx_fast(rl, cl)
                    rl_bc = nrm.tile([64, 512], f32, tag="rlb", name="rlb")
                    nc.gpsimd.partition_broadcast(rl_bc, rl)
                    nc.vector.tensor_tensor(out=ctxn[b][hsl, qsl],
                                            in0=pctx[h][0:64, :], in1=rl_bc,
                                            op=MUL)

            def nothing():
                pass

            # ---- schedule ------------------------------------------------
            # stage A: K and Q of b=0 land on separate HWDGE queues; PE
            # warmup junk keeps the HAM clock at 2.4GHz during the DMAs.
            xk_t = load_x(xk, "k", 0, "sync")
            xq_t = load_x(xq, "q", 0, "scalar")
            for wu in range(40):
                jp = pj.tile([128, 512], f32, tag="pj", name="jp")
                nc.tensor.matmul(jp, w_sb["wq"][:, wu % 8, :],
                                 w_sb["wk"][:, (wu % 2) * 4:(wu % 2) * 4 + 4, :],
                                 start=True, stop=True)
            for qc in range(NQC):
                proj_qk_chunk(0, "k", xk_t, qc)
            proj_qk_chunk(0, "q", xq_t, 0)

            x2 = {}
            # x DMA order per HWDGE queue: b0 {xk|sync, xq|scalar}, then
            # dual-queue halves of xv0, xk1, xq1, xv1 back-to-back. Fillers
            # that depend on a load sit in a later qc's PV phase, where a
            # stall can't starve the ACT exp stream.
            attn_qc(0, 0,
                    s_inj=[lambda: x2.update(v=load_x(xv, "v", 0, "dual")),
                           lambda: proj_qk_chunk(0, "q", xq_t, 1),
                           lambda: proj_qk_chunk(0, "q", xq_t, 2),
                           lambda: proj_qk_chunk(0, "q", xq_t, 3),
                           nothing, nothing, nothing, nothing, nothing,
                           lambda: proj_v_tt(0, x2["v"], [0, 1]),
                           lambda: proj_v_tt(0, x2["v"], [2, 3]),
                           lambda: proj_v_tt(0, x2["v"], [4, 5]),
                           lambda: proj_v_tt(0, x2["v"], [6, 7]),
                           lambda: proj_v_tt(0, x2["v"], [8, 9]),
                           lambda: proj_v_tt(0, x2["v"], [10, 11])],
                    pv_inj=[lambda: x2.update(k=load_x(xk, "k", 1, "dual")),
                            lambda: proj_v_tt(0, x2["v"], [12, 13]),
                            lambda: proj_v_tt(0, x2["v"], [14, 15])])
            attn_qc(0, 1,
                    s_inj=[lambda: x2.update(q=load_x(xq, "q", 1, "dual"))],
                    pv_inj=[lambda: proj_qk_chunk(1, "k", x2["k"], 0),
                            lambda: proj_qk_chunk(1, "k", x2["k"], 1),
                            lambda: proj_qk_chunk(1, "k", x2["k"], 2),
                            lambda: proj_qk_chunk(1, "k", x2["k"], 3)])
            attn_qc(0, 2,
                    s_inj=[lambda: x2.update(v=load_x(xv, "v", 1, "dual"))],
                    pv_inj=[lambda: proj_qk_chunk(1, "q", x2["q"], 0),
                            lambda: proj_qk_chunk(1, "q", x2["q"], 1),
                            lambda: proj_qk_chunk(1, "q", x2["q"], 2),
                            lambda: proj_qk_chunk(1, "q", x2["q"], 3),
                            lambda: outproj_tt(0, [0, 1]),
                            lambda: outproj_tt(0, [2, 3])])
            attn_qc(0, 3,
                    pv_inj=[lambda: proj_v_tt(1, x2["v"], [0, 1]),
                            lambda: proj_v_tt(1, x2["v"], [2, 3]),
                            lambda: proj_v_tt(1, x2["v"], [4, 5]),
                            lambda: proj_v_tt(1, x2["v"], [6, 7]),
                            lambda: proj_v_tt(1, x2["v"], [8, 9]),
                            lambda: proj_v_tt(1, x2["v"], [10, 11]),
                            lambda: proj_v_tt(1, x2["v"], [12, 13]),
                            lambda: proj_v_tt(1, x2["v"], [14, 15])])
            attn_qc(1, 0,
                    pv_inj=[lambda: outproj_tt(0, [4, 5]),
                            lambda: outproj_tt(0, [6, 7]),
                            lambda: outproj_tt(0, [8, 9]),
                            lambda: outproj_tt(0, [10, 11])])
            attn_qc(1, 1,
                    pv_inj=[lambda: outproj_tt(0, [12, 13]),
                            lambda: outproj_tt(0, [14, 15]),
                            lambda: outproj_tt(1, [0, 1]),
                            lambda: outproj_tt(1, [2, 3])])
            attn_qc(1, 2,
                    pv_inj=[lambda: outproj_tt(1, [4, 5]),
                            lambda: outproj_tt(1, [6, 7])])
            attn_qc(1, 3,
                    pv_inj=[lambda: outproj_tt(1, [8, 9]),
                            lambda: outproj_tt(1, [10, 11])])
            outproj_tt(1, [12, 13], on_scalar=True)
            outproj_tt(1, [14, 15], on_scalar=True)

    nc.compile()
    return nc


def _get_prog(masked: bool):
    key = masked
    if key not in _prog_cache:
        _prog_cache[key] = _build(masked)
    return _prog_cache[key]


def kernel(query, key, value, mask, Wq, bq, Wk, bk, Wv, bv, Wo, bo):
    from concourse.bass_utils import run_bass_kernel_spmd

    query = np.asarray(query)
    key = np.asarray(key)
    value = np.asarray(value)
    mask = np.asarray(mask)
    Wq, bq = np.asarray(Wq), np.asarray(bq)
    Wk, bk = np.asarray(Wk), np.asarray(bk)
    Wv, bv = np.asarray(Wv), np.asarray(bv)
    Wo, bo = np.asarray(Wo), np.asarray(bo)

    masked = not bool(mask.all())
    nc = _get_prog(masked)

    def t16(x):  # [S, B, D] -> contiguous [D, B, S] fp16
        return np.ascontiguousarray(x.transpose(2, 1, 0).astype(np.float16))

    def warr(W, hs):  # [128, KT*128]: row p = concat_kt W[hs+m, kt*128+p]
        wt = W[hs:hs + DLOC, :].T.astype(np.float16)       # [kt*128+p, m]
        return np.ascontiguousarray(
            wt.reshape(KT, 128, DLOC).transpose(1, 0, 2).reshape(128, KT * DLOC))

    xq, xk, xv = t16(query), t16(key), t16(value)
    mb = np.where(mask.reshape(S), 0.0, -1e30).astype(np.float32)

    in_maps = []
    for c in range(NCORES):
        hs = c * DLOC
        in_maps.append({
            "xq": xq, "xk": xk, "xv": xv,
            "wq": warr(Wq, hs),
            "wk": warr(Wk, hs),
            "wv": warr(Wv, hs),
            "wo": np.ascontiguousarray(Wo[:, hs:hs + DLOC].T.astype(np.float16)),
            "bq": bq[hs:hs + DLOC].astype(np.float32),
            "bk": bk[hs:hs + DLOC].astype(np.float32),
            "bv": bv[hs:hs + DLOC].astype(np.float32),
            "mb": mb,
        })

    res = run_bass_kernel_spmd(nc, in_maps, core_ids=list(range(NCORES)))
    acc = res.results[0]["out"].astype(np.float64)
    for c in range(1, NCORES):
        acc += res.results[c]["out"]
    acc += bo.astype(np.float64)
    return acc.astype(np.float32)
